# revision 5
# baseline (speedup 1.0000x reference)
"""GAT 3-layer molecule model on 8 TRN2 NeuronCores (Bass/Tile), fused.

One SPMD launch computes all 3 GAT layers + BN + readout. Nodes are
partitioned into 8 graph-aligned contiguous ranges (one core each); each
core owns its nodes' incoming edges in a degree-sorted ELL layout
(node-per-partition, K slots per 128-node chunk, slot 0 = self loop).

Per layer each core builds only its local [NLOC,264] row table
(xw | asrc | adst) with dense matmuls, AllGathers the full [8*NLOC,264]
table over NeuronLink, then per chunk gathers src rows with one indirect
DMA and runs softmax attention + weighted reduction on DVE. BN statistics
are AllReduced on-device and folded into the next layer's table build, so
the whole model is a single NEFF with no host round trips.

Plan-constant tensors (edge layout, edge attrs, pooling matrix) are kept
resident on device between calls; a warm call ships only x and weights.
"""
import numpy as np

import concourse.bass as bass
import concourse.bacc as bacc
import concourse.mybir as mybir
import concourse.tile as tile

F32 = mybir.dt.float32
BF16 = mybir.dt.bfloat16
I32 = mybir.dt.int32

N, E, F_IN, ED, G, C = 50000, 800000, 32, 10, 512, 64
NCORES = 8
P = 128
NLOC = 6400            # padded local nodes per core (50 chunks)
NCH = NLOC // P        # 50
NTAB = NCORES * NLOC   # 51200 gather-table rows (replica-ordered)
HMAX = 4
ROWW = HMAX * C + 2 * HMAX   # 264: xw(256) | asrc(4) | adst(4)
EPS = 1e-5
NEGB = -1e30

_CACHE = {}


# ----------------------------------------------------------------- host plan
def _make_plan(edge_index, edge_attr, batch):
    src = np.asarray(edge_index[0], dtype=np.int64)
    dst = np.asarray(edge_index[1], dtype=np.int64)
    batch = np.asarray(batch, dtype=np.int64)
    ea = np.asarray(edge_attr, dtype=np.float32)

    # graph-aligned core boundaries
    gstart = np.searchsorted(batch, np.arange(G + 1))  # gstart[G] == N
    bounds = [0]
    for c in range(1, NCORES):
        t = (N * c) // NCORES
        g = int(batch[min(t, N - 1)])
        b0, b1 = int(gstart[g]), int(gstart[min(g + 1, G)])
        bounds.append(b0 if t - b0 <= b1 - t else b1)
    bounds.append(N)

    # edges sorted by dst for grouping
    order_e = np.argsort(dst, kind="stable")
    s_src = src[order_e]
    s_eid = order_e
    deg_all = np.bincount(dst, minlength=N)
    rowptr = np.concatenate([[0], np.cumsum(deg_all)])

    cores = []
    for c in range(NCORES):
        n0, n1 = bounds[c], bounds[c + 1]
        nloc = n1 - n0
        assert nloc <= NLOC, (c, nloc)
        deg = deg_all[n0:n1]
        order = np.argsort(-deg, kind="stable")  # degree-sorted local perm
        cores.append(dict(n0=n0, n1=n1, nloc=nloc, deg=deg, order=order))

    # replica-ordered table position of each global node
    posmap = np.zeros(N, dtype=np.int64)
    for c, cd in enumerate(cores):
        nloc = cd["nloc"]
        posmap[cd["n0"] + cd["order"][:nloc]] = c * NLOC + np.arange(nloc)

    # unified chunk widths across cores
    Ks = []
    for ch in range(NCH):
        m = 0
        for cd in cores:
            dsorted = cd["deg"][cd["order"]]
            sl = dsorted[ch * P:(ch + 1) * P]
            if len(sl):
                m = max(m, int(sl.max()))
        Ks.append(1 + m)
    offs = np.concatenate([[0], np.cumsum(Ks)]).astype(np.int64)
    KTOT = int(offs[-1])

    lp_all = np.arange(NLOC)
    ch_all = lp_all // P
    p_all = lp_all % P
    for cd in cores:
        n0, nloc, deg, order = cd["n0"], cd["nloc"], cd["deg"], cd["order"]
        gidx = np.zeros((P, KTOT), dtype=np.int32)
        eab = np.zeros((P, KTOT, ED + 1), dtype=np.float32)
        eab[:, :, ED] = NEGB                      # default: pad slot
        deginv = np.zeros((P, NCH), dtype=np.float32)
        nmask = np.zeros((P, NCH), dtype=np.float32)
        o_all = offs[ch_all]
        eab[p_all, o_all, ED] = 0.0               # self slot always live
        lp = lp_all[:nloc]
        ch, p, o = ch_all[:nloc], p_all[:nloc], o_all[:nloc]
        n_glob = n0 + order[:nloc]
        gidx[p, o] = posmap[n_glob]
        d = deg[order[:nloc]].astype(np.int64)
        e0 = rowptr[n_glob]
        tot = int(d.sum())
        erow = np.repeat(p, d)
        # per-edge within-node rank: arange minus each node's start
        starts = np.concatenate([[0], np.cumsum(d)[:-1]])
        rank = np.arange(tot) - np.repeat(starts, d)
        ecol = np.repeat(o, d) + 1 + rank
        eidx = np.repeat(e0, d) + rank            # into dst-sorted edge list
        gidx[erow, ecol] = posmap[s_src[eidx]]
        eab[erow, ecol, :ED] = ea[s_eid[eidx]]
        eab[erow, ecol, ED] = 0.0
        deginv[p, ch] = 1.0 / np.maximum(d, 1)
        nmask[p, ch] = 1.0
        cd["gidx"] = gidx
        cd["eab"] = eab.reshape(P, KTOT, ED + 1, 1)
        cd["deginv"] = deginv
        cd["nmask"] = nmask
        cd["xsel"] = (n0 + order[:nloc]).astype(np.int64)
        g0 = int(batch[cd["n0"]]) if nloc else 0
        cd["g0"] = g0
        cd["ng"] = (int(batch[cd["n1"] - 1]) - g0 + 1) if nloc else 0

    GCP = max(max(cd["ng"] for cd in cores), 2)
    GCP = ((GCP + 1) // 2) * 2
    cnt = np.bincount(batch, minlength=G).astype(np.float32)
    for cd in cores:
        PT = np.zeros((P, NCH, GCP), dtype=np.float32)
        nloc, order, n0, g0 = cd["nloc"], cd["order"], cd["n0"], cd["g0"]
        g = batch[n0 + order[:nloc]] - g0
        PT[p_all[:nloc], ch_all[:nloc], g] = (
            1.0 / np.maximum(cnt[g0 + g], 1.0))
        cd["PT"] = PT
    return dict(bounds=bounds, cores=cores, Ks=Ks, offs=offs, KTOT=KTOT,
                GCP=GCP)


# ------------------------------------------------------------ fused builder
def _build_fused(Ks, KTOT, GCP):
    nc = bacc.Bacc(None, target_bir_lowering=False, debug=False,
                   num_devices=NCORES)
    FINS = [F_IN, C, C]
    HS = [4, 2, 4]
    xT = nc.declare_dram_parameter("xT", [F_IN, NLOC], F32, isOutput=False)
    wcat = [nc.declare_dram_parameter(f"wcat{l}", [FINS[l], ROWW], F32,
                                      isOutput=False) for l in range(3)]
    wae = [nc.declare_dram_parameter(f"wae{l}", [P, 1, ED, HMAX], F32,
                                     isOutput=False) for l in range(3)]
    # BN affine params: col layout for layers 0,1 (used partition-wise in the
    # next table build), row layout for layer 2 (used in readout).
    gcol = [nc.declare_dram_parameter(f"gcol{l}", [C, 1], F32,
                                      isOutput=False) for l in range(2)]
    becol = [nc.declare_dram_parameter(f"becol{l}", [C, 1], F32,
                                       isOutput=False) for l in range(2)]
    grow = nc.declare_dram_parameter("grow", [1, C], F32, isOutput=False)
    berow = nc.declare_dram_parameter("berow", [1, C], F32, isOutput=False)
    gidx_d = nc.declare_dram_parameter("gidx", [P, KTOT], I32, isOutput=False)
    eab_d = nc.declare_dram_parameter("eab", [P, KTOT, ED + 1, 1], F32,
                                      isOutput=False)
    deginv_d = nc.declare_dram_parameter("deginv", [P, NCH], F32,
                                         isOutput=False)
    nmask_d = nc.declare_dram_parameter("nmask", [P, NCH], F32,
                                        isOutput=False)
    PT_d = nc.declare_dram_parameter("PT", [P, NCH, GCP], F32, isOutput=False)
    fw1 = nc.declare_dram_parameter("fw1", [C, C], F32, isOutput=False)
    fb1 = nc.declare_dram_parameter("fb1", [C, 1], F32, isOutput=False)
    fw2 = nc.declare_dram_parameter("fw2", [C, 1], F32, isOutput=False)
    out_g = nc.declare_dram_parameter("out_g", [1, GCP], F32, isOutput=True)

    offs = np.concatenate([[0], np.cumsum(Ks)]).astype(int)
    MU = mybir.AluOpType.mult
    AD = mybir.AluOpType.add
    MX = mybir.AluOpType.max
    SUB = mybir.AluOpType.subtract
    INVN = 1.0 / N

    from concourse.masks import make_identity
    with tile.TileContext(nc) as tc:
        with (
            tc.tile_pool(name="const", bufs=1) as cpool,
            tc.tile_pool(name="tb", bufs=2) as tbpool,
            tc.tile_pool(name="tbp", bufs=2, space="PSUM") as tbps,
            tc.tile_pool(name="gath", bufs=2) as gpool,
            tc.tile_pool(name="work", bufs=2) as wpool,
            tc.tile_pool(name="small", bufs=2) as spool,
            tc.tile_pool(name="dram", bufs=1, space="DRAM") as dpool,
        ):
            # ---- persistent constants
            gidx_sb = cpool.tile([P, KTOT], I32)
            nc.sync.dma_start(out=gidx_sb[:], in_=gidx_d[:, :])
            deginv_sb = cpool.tile([P, NCH], F32)
            nmask_sb = cpool.tile([P, NCH], F32)
            nc.sync.dma_start(out=deginv_sb[:], in_=deginv_d[:, :])
            nc.sync.dma_start(out=nmask_sb[:], in_=nmask_d[:, :])
            ident = cpool.tile([P, P], F32)
            make_identity(nc, ident)
            ones_row = cpool.tile([1, P], F32)
            nc.vector.memset(ones_row[:], 1.0)
            ones_col = cpool.tile([P, 1], F32)
            nc.vector.memset(ones_col[:], 1.0)
            hT = cpool.tile([C, NLOC], F32)       # channel-major activations
            h3 = cpool.tile([P, NCH * C], F32)    # layer-3 out, node-major

            w_sb = []
            wae_sb = []
            for l in range(3):
                w_sb.append(cpool.tile([FINS[l], ROWW], F32,
                                       tag=f"wsb{l}", name=f"wsb{l}"))
                nc.sync.dma_start(out=w_sb[l][:], in_=wcat[l][:, :])
                wae_sb.append(cpool.tile([P, 1, ED, HMAX], F32,
                                         tag=f"waesb{l}",
                                         name=f"waesb{l}"))
                nc.sync.dma_start(out=wae_sb[l][:], in_=wae[l][:, :, :, :])
            gcol_sb, becol_sb = [], []
            for l in range(2):
                gcol_sb.append(cpool.tile([C, 1], F32, tag=f"gc{l}",
                                           name=f"gc{l}"))
                nc.sync.dma_start(out=gcol_sb[l][:], in_=gcol[l][:, :])
                becol_sb.append(cpool.tile([C, 1], F32, tag=f"bc{l}",
                                            name=f"bc{l}"))
                nc.sync.dma_start(out=becol_sb[l][:], in_=becol[l][:, :])
            grow_sb = cpool.tile([1, C], F32)
            berow_sb = cpool.tile([1, C], F32)
            nc.sync.dma_start(out=grow_sb[:], in_=grow[:, :])
            nc.sync.dma_start(out=berow_sb[:], in_=berow[:, :])
            fw1_sb = cpool.tile([C, C], F32)
            fb1_sb = cpool.tile([C, 1], F32)
            fw2_sb = cpool.tile([C, 1], F32)
            nc.sync.dma_start(out=fw1_sb[:], in_=fw1[:, :])
            nc.sync.dma_start(out=fb1_sb[:], in_=fb1[:, :])
            nc.sync.dma_start(out=fw2_sb[:], in_=fw2[:, :])

            bnA_col = [None, None]   # set after layers 0,1
            bnB_col = [None, None]
            bn_bc = None             # [P, 2C] row-broadcast bn for readout

            for l in range(3):
                fin = FINS[l]
                tabloc = dpool.tile([NLOC, ROWW], BF16, tag=f"tabloc{l}")
                table = dpool.tile([NTAB, ROWW], BF16, tag=f"table{l}")

                # ---- phase 1: local table rows
                for ch in range(NCH):
                    if l == 0:
                        slab = tbpool.tile([F_IN, P], F32, tag="xslab")
                        nc.sync.dma_start(out=slab[:],
                                          in_=xT[:, ch * P:(ch + 1) * P])
                    else:
                        slab = tbpool.tile([C, P], F32, tag="bnslab")
                        nc.vector.tensor_scalar(
                            out=slab[:], in0=hT[:, ch * P:(ch + 1) * P],
                            scalar1=bnA_col[l - 1][:],
                            scalar2=bnB_col[l - 1][:],
                            op0=MU, op1=AD)
                        nc.scalar.activation(
                            slab[:], slab[:],
                            mybir.ActivationFunctionType.Relu)
                    ps = tbps.tile([P, ROWW], F32, space="PSUM", tag="mm")
                    nc.tensor.matmul(ps[:], lhsT=slab[:], rhs=w_sb[l][:],
                                     start=True, stop=True)
                    rows = tbpool.tile([P, ROWW], BF16, tag="rows")
                    nc.vector.tensor_copy(out=rows[:], in_=ps[:])
                    nc.sync.dma_start(out=tabloc[ch * P:(ch + 1) * P, :],
                                      in_=rows[:])

                # ---- all-gather table across cores
                nc.gpsimd.collective_compute(
                    "AllGather", mybir.AluOpType.bypass,
                    replica_groups=[list(range(NCORES))],
                    ins=[tabloc[:].opt()], outs=[table[:].opt()])

                # ---- phase 2: attention + aggregation per chunk
                ssum = cpool.tile([P, C], F32, tag=f"ssum{l}")
                ssq = cpool.tile([P, C], F32, tag=f"ssq{l}")
                nc.vector.memset(ssum[:], 0.0)
                nc.vector.memset(ssq[:], 0.0)
                for ch in range(NCH):
                    K = int(Ks[ch])
                    o = int(offs[ch])
                    gt = gpool.tile([P, K, ROWW], BF16, tag="gt")
                    for k in range(K):
                        nc.gpsimd.indirect_dma_start(
                            out=gt[:, k, :], out_offset=None,
                            in_=table[:, :],
                            in_offset=bass.IndirectOffsetOnAxis(
                                ap=gidx_sb[:, o + k:o + k + 1], axis=0))
                    ea_t = wpool.tile([P, K, ED + 1, 1], F32, tag="ea")
                    nc.sync.dma_start(out=ea_t[:],
                                      in_=eab_d[:, o:o + K, :, :])

                    # aedge[p,k,h] = sum_d ea[p,k,d] * wae[d,h]
                    prod = wpool.tile([P, K, ED, HMAX], F32, tag="prod")
                    nc.vector.tensor_tensor(
                        out=prod[:],
                        in0=ea_t[:, :, :ED, :].to_broadcast([P, K, ED, HMAX]),
                        in1=wae_sb[l][:].to_broadcast([P, K, ED, HMAX]),
                        op=MU)
                    ae_r = wpool.tile([P, K, HMAX], F32, tag="aer")
                    nc.vector.reduce_sum(
                        out=ae_r[:],
                        in_=prod[:].rearrange("p k d h -> p k h d"),
                        axis=mybir.AxisListType.X)
                    # self slot aedge = mean of incoming (slots 1..K-1)
                    if K > 1:
                        selfae = spool.tile([P, 1, HMAX], F32, tag="selfae")
                        nc.vector.reduce_sum(
                            out=selfae[:, 0, :],
                            in_=ae_r[:, 1:, :].rearrange("p k h -> p h k"),
                            axis=mybir.AxisListType.X)
                        nc.vector.tensor_scalar(
                            out=selfae[:, 0, :], in0=selfae[:, 0, :],
                            scalar1=deginv_sb[:, ch:ch + 1], scalar2=None,
                            op0=MU)
                        nc.vector.tensor_copy(out=ae_r[:, 0:1, :],
                                              in_=selfae[:])

                    # logits = asrc[src] + adst[dst] + aedge + padbias
                    lg = wpool.tile([P, K, HMAX, 1], F32, tag="lg")
                    lg3 = lg[:, :, :, 0]
                    nc.vector.tensor_tensor(
                        out=lg3, in0=gt[:, :, HMAX * C:HMAX * C + HMAX],
                        in1=ae_r[:], op=AD)
                    nc.vector.tensor_tensor(
                        out=lg3, in0=lg3,
                        in1=gt[:, 0:1, HMAX * C + HMAX:HMAX * C + 2 * HMAX]
                            .to_broadcast([P, K, HMAX]),
                        op=AD)
                    nc.vector.tensor_tensor(
                        out=lg3, in0=lg3,
                        in1=ea_t[:, :, ED:ED + 1, 0]
                            .to_broadcast([P, K, HMAX]),
                        op=AD)
                    # leaky_relu(0.2) then exp
                    lk = spool.tile([P, K, HMAX], F32, tag="lk")
                    nc.vector.tensor_scalar(out=lk[:], in0=lg3,
                                            scalar1=0.2, scalar2=None,
                                            op0=MU)
                    nc.vector.tensor_tensor(out=lg3, in0=lg3, in1=lk[:],
                                            op=MX)
                    nc.scalar.activation(lg3, lg3,
                                         mybir.ActivationFunctionType.Exp)
                    # denom + alpha
                    den = spool.tile([P, 1, HMAX], F32, tag="den")
                    nc.vector.reduce_sum(
                        out=den[:, 0, :],
                        in_=lg3.rearrange("p k h -> p h k"),
                        axis=mybir.AxisListType.X)
                    rec = spool.tile([P, 1, HMAX], F32, tag="rec")
                    nc.vector.reciprocal(out=rec[:, 0, :], in_=den[:, 0, :])
                    nc.vector.tensor_tensor(
                        out=lg3, in0=lg3,
                        in1=rec[:].to_broadcast([P, K, HMAX]), op=MU)

                    # weighted sum over slots for all heads at once:
                    # gt.xw *= alpha (in place), then reduce over k
                    xw4 = gt[:, :, :HMAX * C].rearrange(
                        "p k (h c) -> p k h c", h=HMAX)
                    nc.vector.tensor_tensor(
                        out=xw4, in0=xw4,
                        in1=lg[:].to_broadcast([P, K, HMAX, C]), op=MU)
                    hv = spool.tile([P, HMAX, C], F32, tag="hv")
                    nc.vector.reduce_sum(
                        out=hv[:],
                        in_=gt[:, :, :HMAX * C].rearrange(
                            "p k (h c) -> p h c k", h=HMAX),
                        axis=mybir.AxisListType.X)
                    ht_o = wpool.tile([P, C], F32, tag="hto")
                    nc.vector.tensor_tensor(out=ht_o[:], in0=hv[:, 0, :],
                                            in1=hv[:, 1, :], op=AD)
                    nc.vector.tensor_tensor(out=ht_o[:], in0=ht_o[:],
                                            in1=hv[:, 2, :], op=AD)
                    nc.vector.tensor_tensor(out=ht_o[:], in0=ht_o[:],
                                            in1=hv[:, 3, :], op=AD)
                    nc.vector.tensor_scalar(out=ht_o[:], in0=ht_o[:],
                                            scalar1=nmask_sb[:, ch:ch + 1],
                                            scalar2=None, op0=MU)
                    nc.vector.tensor_tensor(out=ssum[:], in0=ssum[:],
                                            in1=ht_o[:], op=AD)
                    sq = wpool.tile([P, C], F32, tag="sq")
                    nc.vector.tensor_tensor(out=sq[:], in0=ht_o[:],
                                            in1=ht_o[:], op=MU)
                    nc.vector.tensor_tensor(out=ssq[:], in0=ssq[:],
                                            in1=sq[:], op=AD)
                    if l < 2:
                        tp = tbps.tile([C, P], F32, space="PSUM",
                                       tag="mm")
                        nc.tensor.transpose(out=tp[:], in_=ht_o[:],
                                            identity=ident[:])
                        nc.vector.tensor_copy(
                            out=hT[:, ch * P:(ch + 1) * P], in_=tp[:])
                    else:
                        nc.vector.tensor_copy(
                            out=h3[:, ch * C:(ch + 1) * C], in_=ht_o[:])

                # ---- BN stats: partition-reduce then AllReduce
                stat2 = cpool.tile([P, P], F32, tag=f"st2{l}")
                nc.vector.memset(stat2[:], 0.0)
                nc.vector.tensor_copy(out=stat2[:, :C], in_=ssum[:])
                nc.vector.tensor_copy(out=stat2[:, C:2 * C], in_=ssq[:])
                sps = tbps.tile([P, 1], F32, space="PSUM", tag="mm")
                nc.tensor.matmul(sps[:], lhsT=stat2[:], rhs=ones_col[:],
                                 start=True, stop=True)
                sout = cpool.tile([P, 1], F32, tag=f"sout{l}")
                nc.vector.tensor_copy(out=sout[:], in_=sps[:])
                statloc = dpool.tile([P, 1], F32, tag=f"stl{l}")
                statred = dpool.tile([P, 1], F32, tag=f"str{l}")
                nc.sync.dma_start(out=statloc[:], in_=sout[:])
                nc.gpsimd.collective_compute(
                    "AllReduce", mybir.AluOpType.add,
                    replica_groups=[list(range(NCORES))],
                    ins=[statloc[:].opt()], outs=[statred[:].opt()])

                epsH2 = EPS * HS[l] * HS[l]
                if l < 2:
                    ssum_c = spool.tile([C, 1], F32, tag="ssc")
                    ssq_c = spool.tile([C, 1], F32, tag="sqc")
                    nc.sync.dma_start(out=ssum_c[:], in_=statred[0:C, :])
                    nc.sync.dma_start(out=ssq_c[:], in_=statred[C:2 * C, :])
                    mu = spool.tile([C, 1], F32, tag="mu")
                    nc.vector.tensor_scalar(out=mu[:], in0=ssum_c[:],
                                            scalar1=INVN, scalar2=None,
                                            op0=MU)
                    var = spool.tile([C, 1], F32, tag="var")
                    nc.vector.tensor_scalar(out=var[:], in0=ssq_c[:],
                                            scalar1=INVN, scalar2=None,
                                            op0=MU)
                    mu2 = spool.tile([C, 1], F32, tag="mu2")
                    nc.vector.tensor_tensor(out=mu2[:], in0=mu[:],
                                            in1=mu[:], op=MU)
                    nc.vector.tensor_tensor(out=var[:], in0=var[:],
                                            in1=mu2[:], op=SUB)
                    nc.vector.tensor_scalar(out=var[:], in0=var[:],
                                            scalar1=epsH2, scalar2=None,
                                            op0=AD)
                    nc.scalar.activation(var[:], var[:],
                                         mybir.ActivationFunctionType.Sqrt)
                    nc.vector.reciprocal(out=var[:], in_=var[:])
                    bnA = cpool.tile([C, 1], F32, tag=f"bnA{l}")
                    nc.vector.tensor_tensor(out=bnA[:], in0=gcol_sb[l][:],
                                            in1=var[:], op=MU)
                    muA = spool.tile([C, 1], F32, tag="muA")
                    nc.vector.tensor_tensor(out=muA[:], in0=mu[:],
                                            in1=bnA[:], op=MU)
                    bnB = cpool.tile([C, 1], F32, tag=f"bnB{l}")
                    nc.vector.tensor_tensor(out=bnB[:], in0=becol_sb[l][:],
                                            in1=muA[:], op=SUB)
                    bnA_col[l] = bnA
                    bnB_col[l] = bnB
                else:
                    srow = spool.tile([1, 2 * C], F32, tag="srow")
                    nc.sync.dma_start(out=srow[:], in_=statred[:, :])
                    mu_r = spool.tile([1, C], F32, tag="mur")
                    nc.vector.tensor_scalar(out=mu_r[:],
                                            in0=srow[:, 0:C],
                                            scalar1=INVN, scalar2=None,
                                            op0=MU)
                    var_r = spool.tile([1, C], F32, tag="varr")
                    nc.vector.tensor_scalar(out=var_r[:],
                                            in0=srow[:, C:2 * C],
                                            scalar1=INVN, scalar2=None,
                                            op0=MU)
                    mu2_r = spool.tile([1, C], F32, tag="mu2r")
                    nc.vector.tensor_tensor(out=mu2_r[:], in0=mu_r[:],
                                            in1=mu_r[:], op=MU)
                    nc.vector.tensor_tensor(out=var_r[:], in0=var_r[:],
                                            in1=mu2_r[:], op=SUB)
                    nc.vector.tensor_scalar(out=var_r[:], in0=var_r[:],
                                            scalar1=epsH2, scalar2=None,
                                            op0=AD)
                    nc.scalar.activation(var_r[:], var_r[:],
                                         mybir.ActivationFunctionType.Sqrt)
                    nc.vector.reciprocal(out=var_r[:], in_=var_r[:])
                    bnAB_r = spool.tile([1, 2 * C], F32, tag="bnabr")
                    nc.vector.tensor_tensor(out=bnAB_r[:, 0:C],
                                            in0=grow_sb[:],
                                            in1=var_r[:], op=MU)
                    muA_r = spool.tile([1, C], F32, tag="muar")
                    nc.vector.tensor_tensor(out=muA_r[:], in0=mu_r[:],
                                            in1=bnAB_r[:, 0:C], op=MU)
                    nc.vector.tensor_tensor(out=bnAB_r[:, C:2 * C],
                                            in0=berow_sb[:],
                                            in1=muA_r[:], op=SUB)
                    # broadcast over the 128 node partitions via ones-matmul
                    bcp = tbps.tile([P, 2 * C], F32, space="PSUM",
                                    tag="mm")
                    nc.tensor.matmul(bcp[:], lhsT=ones_row[:],
                                     rhs=bnAB_r[:], start=True, stop=True)
                    bn_bc = cpool.tile([P, 2 * C], F32)
                    nc.vector.tensor_copy(out=bn_bc[:], in_=bcp[:])

            # ---- readout: bn3 + leaky(0.01) + mean-pool + MLP
            pool_ps = tbps.tile([GCP, C], F32, space="PSUM",
                                tag="poolps", bufs=1)
            for ch in range(NCH):
                hch = wpool.tile([P, C], F32, tag="hch")
                nc.vector.tensor_tensor(
                    out=hch[:], in0=h3[:, ch * C:(ch + 1) * C],
                    in1=bn_bc[:, 0:C], op=MU)
                nc.vector.tensor_tensor(out=hch[:], in0=hch[:],
                                        in1=bn_bc[:, C:2 * C], op=AD)
                lk = wpool.tile([P, C], F32, tag="lkro")
                nc.vector.tensor_scalar(out=lk[:], in0=hch[:],
                                        scalar1=0.01, scalar2=None, op0=MU)
                nc.vector.tensor_tensor(out=hch[:], in0=hch[:], in1=lk[:],
                                        op=MX)
                ptch = wpool.tile([P, GCP], F32, tag="ptch")
                nc.sync.dma_start(out=ptch[:], in_=PT_d[:, ch, :])
                nc.tensor.matmul(pool_ps[:], lhsT=ptch[:], rhs=hch[:],
                                 start=(ch == 0), stop=(ch == NCH - 1))

            pooled = cpool.tile([GCP, C], F32)
            nc.vector.tensor_copy(out=pooled[:], in_=pool_ps[:])
            tps2 = tbps.tile([C, GCP], F32, space="PSUM", tag="mm")
            nc.tensor.transpose(out=tps2[:], in_=pooled[:],
                                identity=ident[:GCP, :GCP])
            pooledT = cpool.tile([C, GCP], F32)
            nc.vector.tensor_copy(out=pooledT[:], in_=tps2[:])
            z_ps = tbps.tile([C, GCP], F32, space="PSUM", tag="mm")
            nc.tensor.matmul(z_ps[:], lhsT=fw1_sb[:], rhs=pooledT[:],
                             start=True, stop=True)
            z1 = cpool.tile([C, GCP], F32)
            nc.vector.tensor_scalar(out=z1[:], in0=z_ps[:],
                                    scalar1=fb1_sb[:], scalar2=None, op0=AD)
            nc.scalar.activation(z1[:], z1[:],
                                 mybir.ActivationFunctionType.Relu)
            o_ps = tbps.tile([1, GCP], F32, space="PSUM", tag="mm")
            nc.tensor.matmul(o_ps[:], lhsT=fw2_sb[:], rhs=z1[:],
                             start=True, stop=True)
            o_sb = cpool.tile([1, GCP], F32)
            nc.vector.tensor_copy(out=o_sb[:], in_=o_ps[:])
            nc.sync.dma_start(out=out_g[:, :], in_=o_sb[:])
    nc.finalize()
    return nc


# ---------------------------------------------------------- persistent exec
class _Exec:
    """jit-compiled SPMD executor that keeps chosen inputs device-resident."""

    def __init__(self, nc, devices=None, donate=True):
        import jax
        from jax.sharding import Mesh, PartitionSpec, NamedSharding
        from jax.experimental.shard_map import shard_map
        from concourse import bass2jax as b2j
        b2j.install_neuronx_cc_hook()
        self.jax = jax
        self.nc = nc
        part_name = (nc.partition_id_tensor.name
                     if nc.partition_id_tensor else None)
        in_names, out_names, out_avals, zero_shapes = [], [], [], []
        for alloc in nc.m.functions[0].allocations:
            if not isinstance(alloc, mybir.MemoryLocationSet):
                continue
            name = alloc.memorylocations[0].name
            if alloc.kind == "ExternalInput":
                if name != part_name:
                    in_names.append(name)
            elif alloc.kind == "ExternalOutput":
                out_names.append(name)
                shape = tuple(alloc.tensor_shape)
                dtype = mybir.dt.np(alloc.dtype)
                out_avals.append(jax.core.ShapedArray(shape, dtype))
                zero_shapes.append((shape, dtype))
        self.in_names = list(in_names)
        self.out_names = out_names
        self.out_avals = out_avals
        self.zero_shapes = zero_shapes
        n_params = len(in_names)
        n_outs = len(out_names)
        bind_names = in_names + out_names + ([part_name] if part_name else [])

        def _body(*args):
            operands = list(args)
            if part_name is not None:
                operands.append(b2j.partition_id_tensor())
            outs = b2j._bass_exec_p.bind(
                *operands,
                out_avals=tuple(out_avals),
                in_names=tuple(bind_names),
                out_names=tuple(out_names),
                lowering_input_output_aliases=(),
                sim_require_finite=True,
                sim_require_nnan=True,
                nc=nc,
            )
            return tuple(outs)

        if devices is None:
            devices = jax.devices()[:NCORES]
        assert len(devices) == NCORES
        self.mesh = Mesh(np.asarray(devices), ("core",))
        self.sharding = NamedSharding(self.mesh, PartitionSpec("core"))
        in_specs = (PartitionSpec("core"),) * (n_params + n_outs)
        out_specs = (PartitionSpec("core"),) * n_outs
        donate_idx = (tuple(range(n_params, n_params + n_outs))
                      if donate else ())
        self.fn = jax.jit(
            shard_map(_body, mesh=self.mesh, in_specs=in_specs,
                      out_specs=out_specs, check_rep=False),
            donate_argnums=donate_idx, keep_unused=True)

    def put_const(self, arr):
        """Ship a concatenated per-core array to the device mesh once."""
        return self.jax.device_put(np.ascontiguousarray(arr), self.sharding)

    def run(self, args_by_name):
        args = [args_by_name[n] for n in self.in_names]
        args += [np.zeros((NCORES * s[0], *s[1:]), d)
                 for s, d in self.zero_shapes]
        outs = self.fn(*args)
        res = []
        for i, name in enumerate(self.out_names):
            a = np.asarray(outs[i])
            res.append(a.reshape(NCORES, *self.out_avals[i].shape))
        return dict(zip(self.out_names, res))


# ------------------------------------------------------------------- driver
def _fold_weights(w, a_s, a_d, we, a_e, fin):
    H = a_s.shape[0]
    wp = np.zeros((C, HMAX * C), np.float32)
    wp[:fin, :H * C] = w
    wep = np.zeros((ED, HMAX * C), np.float32)
    wep[:, :H * C] = we

    def pv(v):
        o = np.zeros((HMAX, C), np.float32)
        o[:H] = v
        return o

    asp, adp, aep = pv(a_s), pv(a_d), pv(a_e)
    w3 = wp.reshape(C, HMAX, C)
    W_as = np.einsum('fhc,hc->fh', w3, asp)
    W_ad = np.einsum('fhc,hc->fh', w3, adp)
    wcat_full = np.concatenate([wp, W_as, W_ad], axis=1).astype(np.float32)
    waev = np.einsum('dhc,hc->dh', wep.reshape(ED, HMAX, C), aep)
    wae_rep = np.ascontiguousarray(
        np.broadcast_to(waev.reshape(1, 1, ED, HMAX), (P, 1, ED, HMAX)),
        dtype=np.float32)
    return wcat_full, wae_rep


def _same(a, b):
    if a is b:
        return True
    return (a.shape == b.shape and a.dtype == b.dtype
            and np.array_equal(a, b))


def kernel(**inp):
    import os
    import time as _t
    verbose = bool(os.environ.get("BASS_VERBOSE"))
    t_start = _t.time()
    inp = {k: np.asarray(v) for k, v in inp.items()}

    # ---- plan group: edge_index / batch / edge_attr
    PLAN_KEYS = ("edge_index", "batch", "edge_attr")
    pg = _CACHE.get("plan_group")
    if pg is None or not all(_same(inp[k], pg["in"][k]) for k in PLAN_KEYS):
        t0 = _t.time()
        plan = _make_plan(inp["edge_index"], inp["edge_attr"], inp["batch"])
        if verbose:
            print(f"  plan build {_t.time()-t0:.2f}s", flush=True)
        pg = dict(plan=plan,
                  in_={k: inp[k].copy() for k in PLAN_KEYS})
        pg["in"] = pg.pop("in_")
        _CACHE["plan_group"] = pg
        _CACHE.pop("consts", None)
        _CACHE.pop("x_group", None)
    plan = pg["plan"]
    Ks, KTOT, GCP = plan["Ks"], plan["KTOT"], plan["GCP"]
    cores = plan["cores"]

    nkey = ("fused", KTOT, tuple(Ks), GCP)
    if nkey not in _CACHE:
        t0 = _t.time()
        _CACHE[nkey] = _build_fused(Ks, KTOT, GCP)
        if verbose:
            print(f"  nc build {_t.time()-t0:.2f}s", flush=True)
    nc = _CACHE[nkey]
    ekey = ("exec", nkey)
    if ekey not in _CACHE:
        t0 = _t.time()
        _CACHE[ekey] = _Exec(nc)
        if verbose:
            print(f"  exec init {_t.time()-t0:.2f}s", flush=True)
    ex = _CACHE[ekey]

    # device-resident plan constants (shipped once per plan)
    if "consts" not in _CACHE:
        t0 = _t.time()
        consts = {}
        for name, field in (("gidx", "gidx"), ("eab", "eab"),
                            ("deginv", "deginv"), ("nmask", "nmask"),
                            ("PT", "PT")):
            consts[name] = ex.put_const(
                np.concatenate([cd[field] for cd in cores], axis=0))
        _CACHE["consts"] = consts
        if verbose:
            print(f"  consts put {_t.time()-t0:.2f}s", flush=True)
    consts = _CACHE["consts"]

    # ---- x group
    xg = _CACHE.get("x_group")
    if xg is None or not _same(inp["x"], xg["x"]):
        x = np.asarray(inp["x"], np.float32)
        xT_cat = np.zeros((NCORES * F_IN, NLOC), np.float32)
        for c, cd in enumerate(cores):
            nloc = cd["nloc"]
            xT_cat[c * F_IN:(c + 1) * F_IN, :nloc] = x[cd["xsel"]].T
        xg = dict(x=inp["x"].copy(), dev=ex.put_const(xT_cat))
        _CACHE["x_group"] = xg

    # ---- weights group
    WKEYS = ("w1", "as1", "ad1", "we1", "ae1", "g1", "be1",
             "w2", "as2", "ad2", "we2", "ae2", "g2", "be2",
             "w3", "as3", "ad3", "we3", "ae3", "g3", "be3",
             "fw1", "fb1", "fw2")
    wg = _CACHE.get("w_group")
    if wg is None or not all(_same(inp[k], wg["in"][k]) for k in WKEYS):
        wargs = {}
        lw = [(inp["w1"], inp["as1"], inp["ad1"], inp["we1"], inp["ae1"],
               F_IN),
              (inp["w2"], inp["as2"], inp["ad2"], inp["we2"], inp["ae2"], C),
              (inp["w3"], inp["as3"], inp["ad3"], inp["we3"], inp["ae3"], C)]
        for l, (w, a_s, a_d, we, a_e, fin) in enumerate(lw):
            wcat_full, wae_rep = _fold_weights(
                np.asarray(w, np.float32), np.asarray(a_s, np.float32),
                np.asarray(a_d, np.float32), np.asarray(we, np.float32),
                np.asarray(a_e, np.float32), fin)
            wargs[f"wcat{l}"] = np.concatenate([wcat_full[:fin]] * NCORES,
                                               axis=0)
            wargs[f"wae{l}"] = np.concatenate([wae_rep] * NCORES, axis=0)
        for l, (g, be) in enumerate(((inp["g1"], inp["be1"]),
                                     (inp["g2"], inp["be2"]))):
            gc = np.asarray(g, np.float32).reshape(C, 1)
            bc = np.asarray(be, np.float32).reshape(C, 1)
            wargs[f"gcol{l}"] = np.concatenate([gc] * NCORES, axis=0)
            wargs[f"becol{l}"] = np.concatenate([bc] * NCORES, axis=0)
        g3 = np.asarray(inp["g3"], np.float32).reshape(1, C)
        be3 = np.asarray(inp["be3"], np.float32).reshape(1, C)
        wargs["grow"] = np.concatenate([g3] * NCORES, axis=0)
        wargs["berow"] = np.concatenate([be3] * NCORES, axis=0)
        wargs["fw1"] = np.concatenate(
            [np.asarray(inp["fw1"], np.float32)] * NCORES, axis=0)
        wargs["fb1"] = np.concatenate(
            [np.asarray(inp["fb1"], np.float32).reshape(C, 1)] * NCORES,
            axis=0)
        wargs["fw2"] = np.concatenate(
            [np.asarray(inp["fw2"], np.float32).reshape(C, 1)] * NCORES,
            axis=0)
        wg = dict(in_={k: inp[k].copy() for k in WKEYS},
                  dev={k: ex.put_const(v) for k, v in wargs.items()})
        wg["in"] = wg.pop("in_")
        _CACHE["w_group"] = wg

    args = dict(consts)
    args["xT"] = xg["dev"]
    args.update(wg["dev"])
    if verbose:
        print(f"  host prep total {_t.time()-t_start:.2f}s", flush=True)

    t0 = _t.time()
    try:
        res = ex.run(args)
        _CACHE.pop("retrying", None)
    except Exception:
        # transient device/backend failure: rebuild executor and
        # device-resident inputs once, then retry
        if "retrying" in _CACHE:
            raise
        _CACHE["retrying"] = True
        for k in ("consts", "x_group", "w_group", ekey):
            _CACHE.pop(k, None)
        import traceback
        traceback.print_exc()
        return kernel(**inp)
    wall = _t.time() - t0
    kernel.launch_walls = [wall]
    kernel.last_exec_ns = 0.0

    og = res["out_g"].reshape(NCORES, GCP)
    fb2 = float(np.asarray(inp["fb2"]).reshape(-1)[0])
    fb1v = np.asarray(inp["fb1"], np.float32).reshape(-1)
    fw2v = np.asarray(inp["fw2"], np.float32).reshape(-1)
    empty_val = float(np.maximum(fb1v, 0.0) @ fw2v) + fb2
    out = np.full(G, empty_val, np.float32)
    for c, cd in enumerate(cores):
        out[cd["g0"]:cd["g0"] + cd["ng"]] = og[c, :cd["ng"]] + fb2
    return out


kernel.last_exec_ns = 0.0
kernel.launch_walls = []


# revision 7
# speedup vs baseline: 1.0012x; 1.0012x over previous
"""GAT 3-layer molecule model on 8 TRN2 NeuronCores (Bass/Tile), fused.

One SPMD launch computes all 3 GAT layers + BN + readout. Nodes are
partitioned into 8 graph-aligned contiguous ranges (one core each); each
core owns its nodes' incoming edges in a degree-sorted ELL layout
(node-per-partition, K slots per 128-node chunk, slot 0 = self loop).

Per layer each core builds only its local [NLOC,264] row table
(xw | asrc | adst) with dense matmuls, AllGathers the full [8*NLOC,264]
table over NeuronLink, then per chunk gathers src rows with one indirect
DMA and runs softmax attention + weighted reduction on DVE. BN statistics
are AllReduced on-device and folded into the next layer's table build, so
the whole model is a single NEFF with no host round trips.

Plan-constant tensors (edge layout, edge attrs, pooling matrix) are kept
resident on device between calls; a warm call ships only x and weights.
"""
import numpy as np

import concourse.bass as bass
import concourse.bacc as bacc
import concourse.mybir as mybir
import concourse.tile as tile

F32 = mybir.dt.float32
BF16 = mybir.dt.bfloat16
I32 = mybir.dt.int32

N, E, F_IN, ED, G, C = 50000, 800000, 32, 10, 512, 64
NCORES = 8
P = 128
NLOC = 6400            # padded local nodes per core (50 chunks)
NCH = NLOC // P        # 50
NTAB = NCORES * NLOC   # 51200 gather-table rows (replica-ordered)
HMAX = 4
ROWW = HMAX * C + 2 * HMAX   # 264: xw(256) | asrc(4) | adst(4)
EPS = 1e-5
NEGB = -1e30

_CACHE = {}


# ----------------------------------------------------------------- host plan
def _make_plan(edge_index, edge_attr, batch):
    src = np.asarray(edge_index[0], dtype=np.int64)
    dst = np.asarray(edge_index[1], dtype=np.int64)
    batch = np.asarray(batch, dtype=np.int64)
    ea = np.asarray(edge_attr, dtype=np.float32)

    # graph-aligned core boundaries
    gstart = np.searchsorted(batch, np.arange(G + 1))  # gstart[G] == N
    bounds = [0]
    for c in range(1, NCORES):
        t = (N * c) // NCORES
        g = int(batch[min(t, N - 1)])
        b0, b1 = int(gstart[g]), int(gstart[min(g + 1, G)])
        bounds.append(b0 if t - b0 <= b1 - t else b1)
    bounds.append(N)

    # edges sorted by dst for grouping
    order_e = np.argsort(dst, kind="stable")
    s_src = src[order_e]
    s_eid = order_e
    deg_all = np.bincount(dst, minlength=N)
    rowptr = np.concatenate([[0], np.cumsum(deg_all)])

    cores = []
    for c in range(NCORES):
        n0, n1 = bounds[c], bounds[c + 1]
        nloc = n1 - n0
        assert nloc <= NLOC, (c, nloc)
        deg = deg_all[n0:n1]
        order = np.argsort(-deg, kind="stable")  # degree-sorted local perm
        cores.append(dict(n0=n0, n1=n1, nloc=nloc, deg=deg, order=order))

    # replica-ordered table position of each global node
    posmap = np.zeros(N, dtype=np.int64)
    for c, cd in enumerate(cores):
        nloc = cd["nloc"]
        posmap[cd["n0"] + cd["order"][:nloc]] = c * NLOC + np.arange(nloc)

    # unified chunk widths across cores
    Ks = []
    for ch in range(NCH):
        m = 0
        for cd in cores:
            dsorted = cd["deg"][cd["order"]]
            sl = dsorted[ch * P:(ch + 1) * P]
            if len(sl):
                m = max(m, int(sl.max()))
        Ks.append(1 + m)
    offs = np.concatenate([[0], np.cumsum(Ks)]).astype(np.int64)
    KTOT = int(offs[-1])

    lp_all = np.arange(NLOC)
    ch_all = lp_all // P
    p_all = lp_all % P
    for cd in cores:
        n0, nloc, deg, order = cd["n0"], cd["nloc"], cd["deg"], cd["order"]
        gidx = np.zeros((P, KTOT), dtype=np.int32)
        eab = np.zeros((P, KTOT, ED + 1), dtype=np.float32)
        eab[:, :, ED] = NEGB                      # default: pad slot
        deginv = np.zeros((P, NCH), dtype=np.float32)
        nmask = np.zeros((P, NCH), dtype=np.float32)
        o_all = offs[ch_all]
        eab[p_all, o_all, ED] = 0.0               # self slot always live
        lp = lp_all[:nloc]
        ch, p, o = ch_all[:nloc], p_all[:nloc], o_all[:nloc]
        n_glob = n0 + order[:nloc]
        gidx[p, o] = posmap[n_glob]
        d = deg[order[:nloc]].astype(np.int64)
        e0 = rowptr[n_glob]
        tot = int(d.sum())
        erow = np.repeat(p, d)
        # per-edge within-node rank: arange minus each node's start
        starts = np.concatenate([[0], np.cumsum(d)[:-1]])
        rank = np.arange(tot) - np.repeat(starts, d)
        ecol = np.repeat(o, d) + 1 + rank
        eidx = np.repeat(e0, d) + rank            # into dst-sorted edge list
        gidx[erow, ecol] = posmap[s_src[eidx]]
        eab[erow, ecol, :ED] = ea[s_eid[eidx]]
        eab[erow, ecol, ED] = 0.0
        deginv[p, ch] = 1.0 / np.maximum(d, 1)
        nmask[p, ch] = 1.0
        cd["gidx"] = gidx
        cd["eab"] = eab.reshape(P, KTOT, ED + 1, 1)
        cd["deginv"] = deginv
        cd["nmask"] = nmask
        cd["xsel"] = (n0 + order[:nloc]).astype(np.int64)
        g0 = int(batch[cd["n0"]]) if nloc else 0
        cd["g0"] = g0
        cd["ng"] = (int(batch[cd["n1"] - 1]) - g0 + 1) if nloc else 0

    GCP = max(max(cd["ng"] for cd in cores), 2)
    GCP = ((GCP + 1) // 2) * 2
    cnt = np.bincount(batch, minlength=G).astype(np.float32)
    for cd in cores:
        PT = np.zeros((P, NCH, GCP), dtype=np.float32)
        nloc, order, n0, g0 = cd["nloc"], cd["order"], cd["n0"], cd["g0"]
        g = batch[n0 + order[:nloc]] - g0
        PT[p_all[:nloc], ch_all[:nloc], g] = (
            1.0 / np.maximum(cnt[g0 + g], 1.0))
        cd["PT"] = PT
    return dict(bounds=bounds, cores=cores, Ks=Ks, offs=offs, KTOT=KTOT,
                GCP=GCP)


# ------------------------------------------------------------ fused builder
def _build_fused(Ks, KTOT, GCP):
    nc = bacc.Bacc(None, target_bir_lowering=False, debug=False,
                   num_devices=NCORES)
    FINS = [F_IN, C, C]
    HS = [4, 2, 4]
    xT = nc.declare_dram_parameter("xT", [F_IN, NLOC], F32, isOutput=False)
    wcat = [nc.declare_dram_parameter(f"wcat{l}", [FINS[l], ROWW], F32,
                                      isOutput=False) for l in range(3)]
    wae = [nc.declare_dram_parameter(f"wae{l}", [P, 1, ED, HMAX], F32,
                                     isOutput=False) for l in range(3)]
    # BN affine params: col layout for layers 0,1 (used partition-wise in the
    # next table build), row layout for layer 2 (used in readout).
    gcol = [nc.declare_dram_parameter(f"gcol{l}", [C, 1], F32,
                                      isOutput=False) for l in range(2)]
    becol = [nc.declare_dram_parameter(f"becol{l}", [C, 1], F32,
                                       isOutput=False) for l in range(2)]
    grow = nc.declare_dram_parameter("grow", [1, C], F32, isOutput=False)
    berow = nc.declare_dram_parameter("berow", [1, C], F32, isOutput=False)
    gidx_d = nc.declare_dram_parameter("gidx", [P, KTOT], I32, isOutput=False)
    eab_d = nc.declare_dram_parameter("eab", [P, KTOT, ED + 1, 1], F32,
                                      isOutput=False)
    deginv_d = nc.declare_dram_parameter("deginv", [P, NCH], F32,
                                         isOutput=False)
    nmask_d = nc.declare_dram_parameter("nmask", [P, NCH], F32,
                                        isOutput=False)
    PT_d = nc.declare_dram_parameter("PT", [P, NCH, GCP], F32, isOutput=False)
    fw1 = nc.declare_dram_parameter("fw1", [C, C], F32, isOutput=False)
    fb1 = nc.declare_dram_parameter("fb1", [C, 1], F32, isOutput=False)
    fw2 = nc.declare_dram_parameter("fw2", [C, 1], F32, isOutput=False)
    out_g = nc.declare_dram_parameter("out_g", [1, GCP], F32, isOutput=True)

    offs = np.concatenate([[0], np.cumsum(Ks)]).astype(int)
    MU = mybir.AluOpType.mult
    AD = mybir.AluOpType.add
    MX = mybir.AluOpType.max
    SUB = mybir.AluOpType.subtract
    INVN = 1.0 / N

    from concourse.masks import make_identity
    with tile.TileContext(nc) as tc:
        with (
            tc.tile_pool(name="const", bufs=1) as cpool,
            tc.tile_pool(name="tb", bufs=2) as tbpool,
            tc.tile_pool(name="tbp", bufs=2, space="PSUM") as tbps,
            tc.tile_pool(name="gath", bufs=2) as gpool,
            tc.tile_pool(name="work", bufs=2) as wpool,
            tc.tile_pool(name="small", bufs=2) as spool,
            tc.tile_pool(name="dram", bufs=1, space="DRAM") as dpool,
        ):
            # ---- persistent constants
            gidx_sb = cpool.tile([P, KTOT], I32)
            nc.sync.dma_start(out=gidx_sb[:], in_=gidx_d[:, :])
            deginv_sb = cpool.tile([P, NCH], F32)
            nmask_sb = cpool.tile([P, NCH], F32)
            nc.sync.dma_start(out=deginv_sb[:], in_=deginv_d[:, :])
            nc.sync.dma_start(out=nmask_sb[:], in_=nmask_d[:, :])
            ident = cpool.tile([P, P], F32)
            make_identity(nc, ident)
            ones_row = cpool.tile([1, P], F32)
            nc.vector.memset(ones_row[:], 1.0)
            ones_col = cpool.tile([P, 1], F32)
            nc.vector.memset(ones_col[:], 1.0)
            hT = cpool.tile([C, NLOC], F32)       # channel-major activations
            h3 = cpool.tile([P, NCH * C], F32)    # layer-3 out, node-major

            w_sb = []
            wae_sb = []
            for l in range(3):
                w_sb.append(cpool.tile([FINS[l], ROWW], F32,
                                       tag=f"wsb{l}", name=f"wsb{l}"))
                nc.sync.dma_start(out=w_sb[l][:], in_=wcat[l][:, :])
                wae_sb.append(cpool.tile([P, 1, ED, HMAX], F32,
                                         tag=f"waesb{l}",
                                         name=f"waesb{l}"))
                nc.sync.dma_start(out=wae_sb[l][:], in_=wae[l][:, :, :, :])
            gcol_sb, becol_sb = [], []
            for l in range(2):
                gcol_sb.append(cpool.tile([C, 1], F32, tag=f"gc{l}",
                                           name=f"gc{l}"))
                nc.sync.dma_start(out=gcol_sb[l][:], in_=gcol[l][:, :])
                becol_sb.append(cpool.tile([C, 1], F32, tag=f"bc{l}",
                                            name=f"bc{l}"))
                nc.sync.dma_start(out=becol_sb[l][:], in_=becol[l][:, :])
            grow_sb = cpool.tile([1, C], F32)
            berow_sb = cpool.tile([1, C], F32)
            nc.sync.dma_start(out=grow_sb[:], in_=grow[:, :])
            nc.sync.dma_start(out=berow_sb[:], in_=berow[:, :])
            fw1_sb = cpool.tile([C, C], F32)
            fb1_sb = cpool.tile([C, 1], F32)
            fw2_sb = cpool.tile([C, 1], F32)
            nc.sync.dma_start(out=fw1_sb[:], in_=fw1[:, :])
            nc.sync.dma_start(out=fb1_sb[:], in_=fb1[:, :])
            nc.sync.dma_start(out=fw2_sb[:], in_=fw2[:, :])

            bnA_col = [None, None]   # set after layers 0,1
            bnB_col = [None, None]
            bn_bc = None             # [P, 2C] row-broadcast bn for readout

            for l in range(3):
                fin = FINS[l]
                tabloc = dpool.tile([NLOC, ROWW], BF16, tag=f"tabloc{l}")
                table = dpool.tile([NTAB, ROWW], BF16, tag=f"table{l}")

                # ---- phase 1: local table rows
                for ch in range(NCH):
                    if l == 0:
                        slab = tbpool.tile([F_IN, P], F32, tag="xslab")
                        nc.sync.dma_start(out=slab[:],
                                          in_=xT[:, ch * P:(ch + 1) * P])
                    else:
                        slab = tbpool.tile([C, P], F32, tag="bnslab")
                        nc.vector.tensor_scalar(
                            out=slab[:], in0=hT[:, ch * P:(ch + 1) * P],
                            scalar1=bnA_col[l - 1][:],
                            scalar2=bnB_col[l - 1][:],
                            op0=MU, op1=AD)
                        nc.scalar.activation(
                            slab[:], slab[:],
                            mybir.ActivationFunctionType.Relu)
                    ps = tbps.tile([P, ROWW], F32, space="PSUM", tag="mm")
                    nc.tensor.matmul(ps[:], lhsT=slab[:], rhs=w_sb[l][:],
                                     start=True, stop=True)
                    rows = tbpool.tile([P, ROWW], BF16, tag="rows")
                    nc.vector.tensor_copy(out=rows[:], in_=ps[:])
                    nc.sync.dma_start(out=tabloc[ch * P:(ch + 1) * P, :],
                                      in_=rows[:])

                # ---- all-gather table across cores
                nc.gpsimd.collective_compute(
                    "AllGather", mybir.AluOpType.bypass,
                    replica_groups=[list(range(NCORES))],
                    ins=[tabloc[:].opt()], outs=[table[:].opt()])

                # ---- phase 2: attention + aggregation per chunk
                ssum = cpool.tile([P, C], F32, tag=f"ssum{l}")
                ssq = cpool.tile([P, C], F32, tag=f"ssq{l}")
                nc.vector.memset(ssum[:], 0.0)
                nc.vector.memset(ssq[:], 0.0)
                H = HS[l]
                for ch in range(NCH):
                    K = int(Ks[ch])
                    o = int(offs[ch])
                    gt = gpool.tile([P, K, ROWW], BF16, tag="gt")
                    for k in range(K):
                        nc.gpsimd.indirect_dma_start(
                            out=gt[:, k, :], out_offset=None,
                            in_=table[:, :],
                            in_offset=bass.IndirectOffsetOnAxis(
                                ap=gidx_sb[:, o + k:o + k + 1], axis=0))
                    ea_t = wpool.tile([P, K, ED + 1, 1], F32, tag="ea")
                    nc.sync.dma_start(out=ea_t[:],
                                      in_=eab_d[:, o:o + K, :, :])

                    # aedge[p,k,h] = sum_d ea[p,k,d] * wae[d,h]
                    prod = wpool.tile([P, K, ED, HMAX], F32, tag="prod")
                    nc.vector.tensor_tensor(
                        out=prod[:, :, :, :H],
                        in0=ea_t[:, :, :ED, :].to_broadcast([P, K, ED, H]),
                        in1=wae_sb[l][:, :, :, :H]
                            .to_broadcast([P, K, ED, H]),
                        op=MU)
                    ae_r = wpool.tile([P, K, HMAX], F32, tag="aer")
                    nc.vector.reduce_sum(
                        out=ae_r[:, :, :H],
                        in_=prod[:, :, :, :H]
                            .rearrange("p k d h -> p k h d"),
                        axis=mybir.AxisListType.X)
                    # self slot aedge = mean of incoming (slots 1..K-1)
                    if K > 1:
                        selfae = spool.tile([P, 1, HMAX], F32, tag="selfae")
                        nc.vector.reduce_sum(
                            out=selfae[:, 0, :H],
                            in_=ae_r[:, 1:, :H]
                                .rearrange("p k h -> p h k"),
                            axis=mybir.AxisListType.X)
                        nc.vector.tensor_scalar(
                            out=selfae[:, 0, :H], in0=selfae[:, 0, :H],
                            scalar1=deginv_sb[:, ch:ch + 1], scalar2=None,
                            op0=MU)
                        nc.vector.tensor_copy(out=ae_r[:, 0:1, :H],
                                              in_=selfae[:, :, :H])

                    # logits = asrc[src] + adst[dst] + aedge + padbias
                    lg = wpool.tile([P, K, HMAX, 1], F32, tag="lg")
                    lg3 = lg[:, :, :H, 0]
                    nc.vector.tensor_tensor(
                        out=lg3, in0=gt[:, :, HMAX * C:HMAX * C + H],
                        in1=ae_r[:, :, :H], op=AD)
                    nc.vector.tensor_tensor(
                        out=lg3, in0=lg3,
                        in1=gt[:, 0:1, HMAX * C + HMAX:HMAX * C + HMAX + H]
                            .to_broadcast([P, K, H]),
                        op=AD)
                    nc.vector.tensor_tensor(
                        out=lg3, in0=lg3,
                        in1=ea_t[:, :, ED:ED + 1, 0]
                            .to_broadcast([P, K, H]),
                        op=AD)
                    # leaky_relu(0.2) then exp
                    lk = spool.tile([P, K, HMAX], F32, tag="lk")
                    nc.vector.tensor_scalar(out=lk[:, :, :H], in0=lg3,
                                            scalar1=0.2, scalar2=None,
                                            op0=MU)
                    nc.vector.tensor_tensor(out=lg3, in0=lg3,
                                            in1=lk[:, :, :H], op=MX)
                    nc.scalar.activation(lg3, lg3,
                                         mybir.ActivationFunctionType.Exp)
                    # denom + alpha
                    den = spool.tile([P, 1, HMAX], F32, tag="den")
                    nc.vector.reduce_sum(
                        out=den[:, 0, :H],
                        in_=lg3.rearrange("p k h -> p h k"),
                        axis=mybir.AxisListType.X)
                    rec = spool.tile([P, 1, HMAX], F32, tag="rec")
                    nc.vector.reciprocal(out=rec[:, 0, :H],
                                         in_=den[:, 0, :H])
                    nc.vector.tensor_tensor(
                        out=lg3, in0=lg3,
                        in1=rec[:, :, :H].to_broadcast([P, K, H]), op=MU)

                    # weighted sum over slots for all heads at once:
                    # gt.xw *= alpha (in place), then reduce over k
                    xw4 = gt[:, :, :H * C].rearrange(
                        "p k (h c) -> p k h c", h=H)
                    nc.vector.tensor_tensor(
                        out=xw4, in0=xw4,
                        in1=lg[:, :, :H, :].to_broadcast([P, K, H, C]),
                        op=MU)
                    hv = spool.tile([P, HMAX, C], F32, tag="hv")
                    nc.vector.reduce_sum(
                        out=hv[:, :H, :],
                        in_=gt[:, :, :H * C].rearrange(
                            "p k (h c) -> p h c k", h=H),
                        axis=mybir.AxisListType.X)
                    ht_o = wpool.tile([P, C], F32, tag="hto")
                    nc.vector.tensor_tensor(out=ht_o[:], in0=hv[:, 0, :],
                                            in1=hv[:, 1, :], op=AD)
                    for hh in range(2, H):
                        nc.vector.tensor_tensor(out=ht_o[:], in0=ht_o[:],
                                                in1=hv[:, hh, :], op=AD)
                    nc.vector.tensor_scalar(out=ht_o[:], in0=ht_o[:],
                                            scalar1=nmask_sb[:, ch:ch + 1],
                                            scalar2=None, op0=MU)
                    nc.vector.tensor_tensor(out=ssum[:], in0=ssum[:],
                                            in1=ht_o[:], op=AD)
                    sq = wpool.tile([P, C], F32, tag="sq")
                    nc.vector.tensor_tensor(out=sq[:], in0=ht_o[:],
                                            in1=ht_o[:], op=MU)
                    nc.vector.tensor_tensor(out=ssq[:], in0=ssq[:],
                                            in1=sq[:], op=AD)
                    if l < 2:
                        tp = tbps.tile([C, P], F32, space="PSUM",
                                       tag="mm")
                        nc.tensor.transpose(out=tp[:], in_=ht_o[:],
                                            identity=ident[:])
                        nc.vector.tensor_copy(
                            out=hT[:, ch * P:(ch + 1) * P], in_=tp[:])
                    else:
                        nc.vector.tensor_copy(
                            out=h3[:, ch * C:(ch + 1) * C], in_=ht_o[:])

                # ---- BN stats: partition-reduce then AllReduce
                stat2 = cpool.tile([P, P], F32, tag=f"st2{l}")
                nc.vector.memset(stat2[:], 0.0)
                nc.vector.tensor_copy(out=stat2[:, :C], in_=ssum[:])
                nc.vector.tensor_copy(out=stat2[:, C:2 * C], in_=ssq[:])
                sps = tbps.tile([P, 1], F32, space="PSUM", tag="mm")
                nc.tensor.matmul(sps[:], lhsT=stat2[:], rhs=ones_col[:],
                                 start=True, stop=True)
                sout = cpool.tile([P, 1], F32, tag=f"sout{l}")
                nc.vector.tensor_copy(out=sout[:], in_=sps[:])
                statloc = dpool.tile([P, 1], F32, tag=f"stl{l}")
                statg = dpool.tile([NCORES, P], F32, tag=f"stg{l}")
                statred = dpool.tile([P, 1], F32, tag=f"str{l}")
                nc.sync.dma_start(out=statloc[:], in_=sout[:])
                nc.gpsimd.collective_compute(
                    "AllGather", mybir.AluOpType.bypass,
                    replica_groups=[list(range(NCORES))],
                    ins=[statloc[:].opt()], outs=[statg[:].opt()])
                s8 = spool.tile([P, NCORES], F32, tag="s8")
                nc.sync.dma_start(
                    out=s8[:], in_=statg[:, :].rearrange("c p -> p c"))
                sred = spool.tile([P, 1], F32, tag="sred")
                nc.vector.reduce_sum(out=sred[:], in_=s8[:],
                                     axis=mybir.AxisListType.X)
                nc.sync.dma_start(out=statred[:], in_=sred[:])

                epsH2 = EPS * HS[l] * HS[l]
                if l < 2:
                    ssum_c = spool.tile([C, 1], F32, tag="ssc")
                    ssq_c = spool.tile([C, 1], F32, tag="sqc")
                    nc.sync.dma_start(out=ssum_c[:], in_=statred[0:C, :])
                    nc.sync.dma_start(out=ssq_c[:], in_=statred[C:2 * C, :])
                    mu = spool.tile([C, 1], F32, tag="mu")
                    nc.vector.tensor_scalar(out=mu[:], in0=ssum_c[:],
                                            scalar1=INVN, scalar2=None,
                                            op0=MU)
                    var = spool.tile([C, 1], F32, tag="var")
                    nc.vector.tensor_scalar(out=var[:], in0=ssq_c[:],
                                            scalar1=INVN, scalar2=None,
                                            op0=MU)
                    mu2 = spool.tile([C, 1], F32, tag="mu2")
                    nc.vector.tensor_tensor(out=mu2[:], in0=mu[:],
                                            in1=mu[:], op=MU)
                    nc.vector.tensor_tensor(out=var[:], in0=var[:],
                                            in1=mu2[:], op=SUB)
                    nc.vector.tensor_scalar(out=var[:], in0=var[:],
                                            scalar1=epsH2, scalar2=None,
                                            op0=AD)
                    nc.scalar.activation(var[:], var[:],
                                         mybir.ActivationFunctionType.Sqrt)
                    nc.vector.reciprocal(out=var[:], in_=var[:])
                    bnA = cpool.tile([C, 1], F32, tag=f"bnA{l}")
                    nc.vector.tensor_tensor(out=bnA[:], in0=gcol_sb[l][:],
                                            in1=var[:], op=MU)
                    muA = spool.tile([C, 1], F32, tag="muA")
                    nc.vector.tensor_tensor(out=muA[:], in0=mu[:],
                                            in1=bnA[:], op=MU)
                    bnB = cpool.tile([C, 1], F32, tag=f"bnB{l}")
                    nc.vector.tensor_tensor(out=bnB[:], in0=becol_sb[l][:],
                                            in1=muA[:], op=SUB)
                    bnA_col[l] = bnA
                    bnB_col[l] = bnB
                else:
                    srow = spool.tile([1, 2 * C], F32, tag="srow")
                    nc.sync.dma_start(out=srow[:], in_=statred[:, :])
                    mu_r = spool.tile([1, C], F32, tag="mur")
                    nc.vector.tensor_scalar(out=mu_r[:],
                                            in0=srow[:, 0:C],
                                            scalar1=INVN, scalar2=None,
                                            op0=MU)
                    var_r = spool.tile([1, C], F32, tag="varr")
                    nc.vector.tensor_scalar(out=var_r[:],
                                            in0=srow[:, C:2 * C],
                                            scalar1=INVN, scalar2=None,
                                            op0=MU)
                    mu2_r = spool.tile([1, C], F32, tag="mu2r")
                    nc.vector.tensor_tensor(out=mu2_r[:], in0=mu_r[:],
                                            in1=mu_r[:], op=MU)
                    nc.vector.tensor_tensor(out=var_r[:], in0=var_r[:],
                                            in1=mu2_r[:], op=SUB)
                    nc.vector.tensor_scalar(out=var_r[:], in0=var_r[:],
                                            scalar1=epsH2, scalar2=None,
                                            op0=AD)
                    nc.scalar.activation(var_r[:], var_r[:],
                                         mybir.ActivationFunctionType.Sqrt)
                    nc.vector.reciprocal(out=var_r[:], in_=var_r[:])
                    bnAB_r = spool.tile([1, 2 * C], F32, tag="bnabr")
                    nc.vector.tensor_tensor(out=bnAB_r[:, 0:C],
                                            in0=grow_sb[:],
                                            in1=var_r[:], op=MU)
                    muA_r = spool.tile([1, C], F32, tag="muar")
                    nc.vector.tensor_tensor(out=muA_r[:], in0=mu_r[:],
                                            in1=bnAB_r[:, 0:C], op=MU)
                    nc.vector.tensor_tensor(out=bnAB_r[:, C:2 * C],
                                            in0=berow_sb[:],
                                            in1=muA_r[:], op=SUB)
                    # broadcast over the 128 node partitions via ones-matmul
                    bcp = tbps.tile([P, 2 * C], F32, space="PSUM",
                                    tag="mm")
                    nc.tensor.matmul(bcp[:], lhsT=ones_row[:],
                                     rhs=bnAB_r[:], start=True, stop=True)
                    bn_bc = cpool.tile([P, 2 * C], F32)
                    nc.vector.tensor_copy(out=bn_bc[:], in_=bcp[:])

            # ---- readout: bn3 + leaky(0.01) + mean-pool + MLP
            pool_ps = tbps.tile([GCP, C], F32, space="PSUM",
                                tag="poolps", bufs=1)
            for ch in range(NCH):
                hch = wpool.tile([P, C], F32, tag="hch")
                nc.vector.tensor_tensor(
                    out=hch[:], in0=h3[:, ch * C:(ch + 1) * C],
                    in1=bn_bc[:, 0:C], op=MU)
                nc.vector.tensor_tensor(out=hch[:], in0=hch[:],
                                        in1=bn_bc[:, C:2 * C], op=AD)
                lk = wpool.tile([P, C], F32, tag="lkro")
                nc.vector.tensor_scalar(out=lk[:], in0=hch[:],
                                        scalar1=0.01, scalar2=None, op0=MU)
                nc.vector.tensor_tensor(out=hch[:], in0=hch[:], in1=lk[:],
                                        op=MX)
                ptch = wpool.tile([P, GCP], F32, tag="ptch")
                nc.sync.dma_start(out=ptch[:], in_=PT_d[:, ch, :])
                nc.tensor.matmul(pool_ps[:], lhsT=ptch[:], rhs=hch[:],
                                 start=(ch == 0), stop=(ch == NCH - 1))

            pooled = cpool.tile([GCP, C], F32)
            nc.vector.tensor_copy(out=pooled[:], in_=pool_ps[:])
            tps2 = tbps.tile([C, GCP], F32, space="PSUM", tag="mm")
            nc.tensor.transpose(out=tps2[:], in_=pooled[:],
                                identity=ident[:GCP, :GCP])
            pooledT = cpool.tile([C, GCP], F32)
            nc.vector.tensor_copy(out=pooledT[:], in_=tps2[:])
            z_ps = tbps.tile([C, GCP], F32, space="PSUM", tag="mm")
            nc.tensor.matmul(z_ps[:], lhsT=fw1_sb[:], rhs=pooledT[:],
                             start=True, stop=True)
            z1 = cpool.tile([C, GCP], F32)
            nc.vector.tensor_scalar(out=z1[:], in0=z_ps[:],
                                    scalar1=fb1_sb[:], scalar2=None, op0=AD)
            nc.scalar.activation(z1[:], z1[:],
                                 mybir.ActivationFunctionType.Relu)
            o_ps = tbps.tile([1, GCP], F32, space="PSUM", tag="mm")
            nc.tensor.matmul(o_ps[:], lhsT=fw2_sb[:], rhs=z1[:],
                             start=True, stop=True)
            o_sb = cpool.tile([1, GCP], F32)
            nc.vector.tensor_copy(out=o_sb[:], in_=o_ps[:])
            nc.sync.dma_start(out=out_g[:, :], in_=o_sb[:])
    nc.finalize()
    return nc


# ---------------------------------------------------------- persistent exec
class _Exec:
    """jit-compiled SPMD executor that keeps chosen inputs device-resident."""

    def __init__(self, nc, devices=None, donate=True):
        import jax
        from jax.sharding import Mesh, PartitionSpec, NamedSharding
        from jax.experimental.shard_map import shard_map
        from concourse import bass2jax as b2j
        b2j.install_neuronx_cc_hook()
        self.jax = jax
        self.nc = nc
        part_name = (nc.partition_id_tensor.name
                     if nc.partition_id_tensor else None)
        in_names, out_names, out_avals, zero_shapes = [], [], [], []
        for alloc in nc.m.functions[0].allocations:
            if not isinstance(alloc, mybir.MemoryLocationSet):
                continue
            name = alloc.memorylocations[0].name
            if alloc.kind == "ExternalInput":
                if name != part_name:
                    in_names.append(name)
            elif alloc.kind == "ExternalOutput":
                out_names.append(name)
                shape = tuple(alloc.tensor_shape)
                dtype = mybir.dt.np(alloc.dtype)
                out_avals.append(jax.core.ShapedArray(shape, dtype))
                zero_shapes.append((shape, dtype))
        self.in_names = list(in_names)
        self.out_names = out_names
        self.out_avals = out_avals
        self.zero_shapes = zero_shapes
        n_params = len(in_names)
        n_outs = len(out_names)
        bind_names = in_names + out_names + ([part_name] if part_name else [])

        def _body(*args):
            operands = list(args)
            if part_name is not None:
                operands.append(b2j.partition_id_tensor())
            outs = b2j._bass_exec_p.bind(
                *operands,
                out_avals=tuple(out_avals),
                in_names=tuple(bind_names),
                out_names=tuple(out_names),
                lowering_input_output_aliases=(),
                sim_require_finite=True,
                sim_require_nnan=True,
                nc=nc,
            )
            return tuple(outs)

        if devices is None:
            devices = jax.devices()[:NCORES]
        assert len(devices) == NCORES
        self.mesh = Mesh(np.asarray(devices), ("core",))
        self.sharding = NamedSharding(self.mesh, PartitionSpec("core"))
        in_specs = (PartitionSpec("core"),) * (n_params + n_outs)
        out_specs = (PartitionSpec("core"),) * n_outs
        donate_idx = (tuple(range(n_params, n_params + n_outs))
                      if donate else ())
        self.fn = jax.jit(
            shard_map(_body, mesh=self.mesh, in_specs=in_specs,
                      out_specs=out_specs, check_rep=False),
            donate_argnums=donate_idx, keep_unused=True)

    def put_const(self, arr):
        """Ship a concatenated per-core array to the device mesh once."""
        return self.jax.device_put(np.ascontiguousarray(arr), self.sharding)

    def run(self, args_by_name):
        args = [args_by_name[n] for n in self.in_names]
        args += [np.zeros((NCORES * s[0], *s[1:]), d)
                 for s, d in self.zero_shapes]
        outs = self.fn(*args)
        res = []
        for i, name in enumerate(self.out_names):
            a = np.asarray(outs[i])
            res.append(a.reshape(NCORES, *self.out_avals[i].shape))
        return dict(zip(self.out_names, res))


# ------------------------------------------------------------------- driver
def _fold_weights(w, a_s, a_d, we, a_e, fin):
    H = a_s.shape[0]
    wp = np.zeros((C, HMAX * C), np.float32)
    wp[:fin, :H * C] = w
    wep = np.zeros((ED, HMAX * C), np.float32)
    wep[:, :H * C] = we

    def pv(v):
        o = np.zeros((HMAX, C), np.float32)
        o[:H] = v
        return o

    asp, adp, aep = pv(a_s), pv(a_d), pv(a_e)
    w3 = wp.reshape(C, HMAX, C)
    W_as = np.einsum('fhc,hc->fh', w3, asp)
    W_ad = np.einsum('fhc,hc->fh', w3, adp)
    wcat_full = np.concatenate([wp, W_as, W_ad], axis=1).astype(np.float32)
    waev = np.einsum('dhc,hc->dh', wep.reshape(ED, HMAX, C), aep)
    wae_rep = np.ascontiguousarray(
        np.broadcast_to(waev.reshape(1, 1, ED, HMAX), (P, 1, ED, HMAX)),
        dtype=np.float32)
    return wcat_full, wae_rep


def _same(a, b):
    if a is b:
        return True
    return (a.shape == b.shape and a.dtype == b.dtype
            and np.array_equal(a, b))


def kernel(**inp):
    import os
    import time as _t
    verbose = bool(os.environ.get("BASS_VERBOSE"))
    t_start = _t.time()
    inp = {k: np.asarray(v) for k, v in inp.items()}

    # ---- plan group: edge_index / batch / edge_attr
    PLAN_KEYS = ("edge_index", "batch", "edge_attr")
    pg = _CACHE.get("plan_group")
    if pg is None or not all(_same(inp[k], pg["in"][k]) for k in PLAN_KEYS):
        t0 = _t.time()
        plan = _make_plan(inp["edge_index"], inp["edge_attr"], inp["batch"])
        if verbose:
            print(f"  plan build {_t.time()-t0:.2f}s", flush=True)
        pg = dict(plan=plan,
                  in_={k: inp[k].copy() for k in PLAN_KEYS})
        pg["in"] = pg.pop("in_")
        _CACHE["plan_group"] = pg
        _CACHE.pop("consts", None)
        _CACHE.pop("x_group", None)
    plan = pg["plan"]
    Ks, KTOT, GCP = plan["Ks"], plan["KTOT"], plan["GCP"]
    cores = plan["cores"]

    nkey = ("fused", KTOT, tuple(Ks), GCP)
    if nkey not in _CACHE:
        t0 = _t.time()
        _CACHE[nkey] = _build_fused(Ks, KTOT, GCP)
        if verbose:
            print(f"  nc build {_t.time()-t0:.2f}s", flush=True)
    nc = _CACHE[nkey]
    ekey = ("exec", nkey)
    if ekey not in _CACHE:
        t0 = _t.time()
        _CACHE[ekey] = _Exec(nc)
        if verbose:
            print(f"  exec init {_t.time()-t0:.2f}s", flush=True)
    ex = _CACHE[ekey]

    # device-resident plan constants (shipped once per plan)
    if "consts" not in _CACHE:
        t0 = _t.time()
        consts = {}
        for name, field in (("gidx", "gidx"), ("eab", "eab"),
                            ("deginv", "deginv"), ("nmask", "nmask"),
                            ("PT", "PT")):
            consts[name] = ex.put_const(
                np.concatenate([cd[field] for cd in cores], axis=0))
        _CACHE["consts"] = consts
        if verbose:
            print(f"  consts put {_t.time()-t0:.2f}s", flush=True)
    consts = _CACHE["consts"]

    # ---- x group
    xg = _CACHE.get("x_group")
    if xg is None or not _same(inp["x"], xg["x"]):
        x = np.asarray(inp["x"], np.float32)
        xT_cat = np.zeros((NCORES * F_IN, NLOC), np.float32)
        for c, cd in enumerate(cores):
            nloc = cd["nloc"]
            xT_cat[c * F_IN:(c + 1) * F_IN, :nloc] = x[cd["xsel"]].T
        xg = dict(x=inp["x"].copy(), dev=ex.put_const(xT_cat))
        _CACHE["x_group"] = xg

    # ---- weights group
    WKEYS = ("w1", "as1", "ad1", "we1", "ae1", "g1", "be1",
             "w2", "as2", "ad2", "we2", "ae2", "g2", "be2",
             "w3", "as3", "ad3", "we3", "ae3", "g3", "be3",
             "fw1", "fb1", "fw2")
    wg = _CACHE.get("w_group")
    if wg is None or not all(_same(inp[k], wg["in"][k]) for k in WKEYS):
        wargs = {}
        lw = [(inp["w1"], inp["as1"], inp["ad1"], inp["we1"], inp["ae1"],
               F_IN),
              (inp["w2"], inp["as2"], inp["ad2"], inp["we2"], inp["ae2"], C),
              (inp["w3"], inp["as3"], inp["ad3"], inp["we3"], inp["ae3"], C)]
        for l, (w, a_s, a_d, we, a_e, fin) in enumerate(lw):
            wcat_full, wae_rep = _fold_weights(
                np.asarray(w, np.float32), np.asarray(a_s, np.float32),
                np.asarray(a_d, np.float32), np.asarray(we, np.float32),
                np.asarray(a_e, np.float32), fin)
            wargs[f"wcat{l}"] = np.concatenate([wcat_full[:fin]] * NCORES,
                                               axis=0)
            wargs[f"wae{l}"] = np.concatenate([wae_rep] * NCORES, axis=0)
        for l, (g, be) in enumerate(((inp["g1"], inp["be1"]),
                                     (inp["g2"], inp["be2"]))):
            gc = np.asarray(g, np.float32).reshape(C, 1)
            bc = np.asarray(be, np.float32).reshape(C, 1)
            wargs[f"gcol{l}"] = np.concatenate([gc] * NCORES, axis=0)
            wargs[f"becol{l}"] = np.concatenate([bc] * NCORES, axis=0)
        g3 = np.asarray(inp["g3"], np.float32).reshape(1, C)
        be3 = np.asarray(inp["be3"], np.float32).reshape(1, C)
        wargs["grow"] = np.concatenate([g3] * NCORES, axis=0)
        wargs["berow"] = np.concatenate([be3] * NCORES, axis=0)
        wargs["fw1"] = np.concatenate(
            [np.asarray(inp["fw1"], np.float32)] * NCORES, axis=0)
        wargs["fb1"] = np.concatenate(
            [np.asarray(inp["fb1"], np.float32).reshape(C, 1)] * NCORES,
            axis=0)
        wargs["fw2"] = np.concatenate(
            [np.asarray(inp["fw2"], np.float32).reshape(C, 1)] * NCORES,
            axis=0)
        wg = dict(in_={k: inp[k].copy() for k in WKEYS},
                  dev={k: ex.put_const(v) for k, v in wargs.items()})
        wg["in"] = wg.pop("in_")
        _CACHE["w_group"] = wg

    args = dict(consts)
    args["xT"] = xg["dev"]
    args.update(wg["dev"])
    if verbose:
        print(f"  host prep total {_t.time()-t_start:.2f}s", flush=True)

    t0 = _t.time()
    try:
        res = ex.run(args)
        _CACHE.pop("retrying", None)
    except Exception:
        # transient device/backend failure: rebuild executor and
        # device-resident inputs once, then retry
        if "retrying" in _CACHE:
            raise
        _CACHE["retrying"] = True
        for k in ("consts", "x_group", "w_group", ekey):
            _CACHE.pop(k, None)
        import traceback
        traceback.print_exc()
        return kernel(**inp)
    wall = _t.time() - t0
    kernel.launch_walls = [wall]
    kernel.last_exec_ns = 0.0

    og = res["out_g"].reshape(NCORES, GCP)
    fb2 = float(np.asarray(inp["fb2"]).reshape(-1)[0])
    fb1v = np.asarray(inp["fb1"], np.float32).reshape(-1)
    fw2v = np.asarray(inp["fw2"], np.float32).reshape(-1)
    empty_val = float(np.maximum(fb1v, 0.0) @ fw2v) + fb2
    out = np.full(G, empty_val, np.float32)
    for c, cd in enumerate(cores):
        out[cd["g0"]:cd["g0"] + cd["ng"]] = og[c, :cd["ng"]] + fb2
    return out


kernel.last_exec_ns = 0.0
kernel.launch_walls = []
